# revision 1
# baseline (speedup 1.0000x reference)
"""Multi-head self-attention (1x1-conv QKV + attention + 1x1-conv proj) for
Trainium2, distributed over 8 NeuronCores.

Problem (hardcoded shapes):
  x: (4, 256, 64, 64) f32;  DIM=256, heads=8, head_dim=32, S=64*64=4096.

Sharding: 8 cores = 4 batches x 2 query-halves. Every core holds the full
key/value sequence for its batch (so no cross-core reduction is needed) and
computes attention + projection for its half of the queries. The host just
concatenates the per-core outputs.

Per-core layout strategy (everything "transposed", channels on partitions):
  - Q^T, K^T: (head*32+d, seq) so scores can be computed directly as
    S^T[key, q] (keys on partitions). Keys-on-partitions makes E^T = exp(S^T)
    directly usable as the moving operand of the P@V matmul, and the softmax
    denominator rides along that same matmul as a ones-column appended to V
    (row 32 of the PSUM output = sum over keys of E^T).
  - exp runs on the scalar engine straight out of PSUM in (128, cg*2*256)
    batches (the only transcendental engine; it is the wall-clock bottleneck,
    so everything else is arranged to hide under it).
  - Scores / QKV / proj matmuls run as float32r (full-rate fp32 streaming);
    the P@V matmul runs in f16 (E in [0, ~300], V ~ N(0,1)).
"""

import numpy as np

import concourse.bacc as bacc
import concourse.bass as bass
import concourse.tile as tile
from concourse import mybir

C = 256  # channels
H = 8  # heads
D = 32  # head dim
QB = 256  # query block (matmul N) size
F32 = mybir.dt.float32
F32R = mybir.dt.float32r
F16 = mybir.dt.float16

SCALE = float(D) ** -0.5


def build_nc(S, NQ, cg=2, score_dt=F16, qkv_dt=F32R, pv_dt=F16, use_tile_position=True, bcast='gpsimd'):
    """Build the single-core SPMD program.

    S: full key sequence length, NQ: queries this core handles.
    cg: key chunks (128 keys each) per PSUM scores slot / exp() call.
    """
    assert S % 512 == 0 and NQ % QB == 0 and C == 2 * 128
    T = S // 128  # key chunks
    NQB = NQ // QB  # query blocks

    nc = bacc.Bacc("TRN2", target_bir_lowering=False)

    xc_d = nc.dram_tensor("xc", [C, S], qkv_dt, kind="ExternalInput")
    xq_d = nc.dram_tensor("xq", [C, NQ], qkv_dt, kind="ExternalInput")
    wqkvT_d = nc.dram_tensor("wqkvT", [C, 3 * C], qkv_dt, kind="ExternalInput")
    bqkv_d = nc.dram_tensor("bqkv", [3 * C, 1], F32, kind="ExternalInput")
    wpT_d = nc.dram_tensor("wpT", [C, C], qkv_dt, kind="ExternalInput")
    bp_d = nc.dram_tensor("bp", [C, 1], F32, kind="ExternalInput")
    out_d = nc.dram_tensor("out", [C, NQ], F32, kind="ExternalOutput")

    with tile.TileContext(nc) as tc:
        with (
            tc.tile_pool(name="big", bufs=3) as big,
            tc.tile_pool(name="persist", bufs=1) as per,
            tc.tile_pool(name="r", bufs=4) as r_pool,
            tc.tile_pool(name="R", bufs=4) as R_pool,
            tc.tile_pool(name="po", bufs=4) as po_pool,
        ):
            # ---- persistent SBUF tiles ----
            xc_sb = [big.tile([128, S], qkv_dt, tag="big", name="xcsb") for _ in range(2)]
            xq_sb = [per.tile([128, NQ], qkv_dt, tag=f"xq{k}", name=f"xq{k}") for k in range(2)]
            w_sb = [per.tile([128, 3 * C], qkv_dt, tag=f"w{k}", name=f"w{k}") for k in range(2)]
            wp_sb = [per.tile([128, C], qkv_dt, tag=f"wp{k}", name=f"wp{k}") for k in range(2)]
            b_sb = per.tile([128, 6], F32, tag="b", name="b")  # bq0 bq1 bk0 bk1 bv0 bv1
            bp_sb = per.tile([128, 2], F32, tag="bp", name="bp")
            qt_sb = [per.tile([128, NQ], score_dt, tag=f"qt{k}", name=f"qt{k}") for k in range(2)]
            kt_sb = [per.tile([128, S], score_dt, tag=f"kt{k}", name=f"kt{k}") for k in range(2)]
            v_sb = per.tile([128, T, H, D + 1], F16, tag="v", name="v")
            ot_sb = [per.tile([128, NQ], qkv_dt, tag=f"ot{k}", name=f"ot{k}") for k in range(2)]

            for k in range(2):
                nc.sync.dma_start(out=xc_sb[k], in_=xc_d[128 * k : 128 * (k + 1), :])
                nc.sync.dma_start(out=xq_sb[k], in_=xq_d[128 * k : 128 * (k + 1), :])
                nc.sync.dma_start(out=w_sb[k], in_=wqkvT_d[128 * k : 128 * (k + 1), :])
                nc.sync.dma_start(out=wp_sb[k], in_=wpT_d[128 * k : 128 * (k + 1), :])
                nc.sync.dma_start(
                    out=bp_sb[:, k : k + 1], in_=bp_d[128 * k : 128 * (k + 1), :]
                )
            for j in range(6):
                nc.sync.dma_start(
                    out=b_sb[:, j : j + 1], in_=bqkv_d[128 * j : 128 * (j + 1), :]
                )
            # ones column for the softmax-denominator trick
            nc.vector.memset(v_sb[:, :, :, D : D + 1], 1.0)

            # ---- QKV projections ----
            with tc.tile_pool(name="ps_setup", bufs=4, space="PSUM") as ps_setup:
                # Q^T and K^T: (c' on partitions, seq on free), f32 + bias.
                for which, src_sb, dst_sb, ncol, bcol in (
                    (0, xq_sb, qt_sb, NQ, 0),
                    (1, xc_sb, kt_sb, S, 2),
                ):
                    for m in range(2):
                        for n in range(ncol // 512):
                            ps = ps_setup.tile([128, 512], F32, tag="mm", name="psqk")
                            for k in range(2):
                                nc.tensor.matmul(
                                    ps,
                                    w_sb[k][
                                            :,
                                            256 * which
                                            + 128 * m : 256 * which
                                            + 128 * (m + 1),
                                        ]
                                    ,
                                    src_sb[k][:, 512 * n : 512 * (n + 1)],
                                    start=(k == 0),
                                    stop=(k == 1),
                                )
                            nc.vector.tensor_scalar_add(
                                out=dst_sb[m][:, 512 * n : 512 * (n + 1)],
                                in0=ps,
                                scalar1=b_sb[:, bcol + m : bcol + m + 1],
                            )
                # V in natural layout (keys on partitions): (128, t, h, d) f16.
                # v-bias is folded into the post-attention output instead
                # (softmax rows sum to 1).
                for t in range(T):
                    ps = ps_setup.tile([128, 256], F32, tag="mm", name="psv")
                    for k in range(2):
                        nc.tensor.matmul(
                            ps,
                            xc_sb[k][:, 128 * t : 128 * (t + 1)],
                            w_sb[k][:, 512:768],
                            start=(k == 0),
                            stop=(k == 1),
                        )
                    nc.vector.tensor_copy(
                        out=v_sb[:, t, :, 0:D],
                        in_=ps.rearrange("p (h d) -> p h d", h=H),
                    )

            # ---- attention + projection ----
            # One-deep software pipeline: the P@V matmuls of the previous
            # (head-pair, query-block) item are interleaved between the score
            # groups of the current item, so the scalar engine (exp -- the
            # throughput floor) always has fresh scores and the PE stream
            # stays dense (HAM stays un-throttled).
            with (
                tc.tile_pool(name="ps_s", bufs=3, space="PSUM") as ps_s,
                tc.tile_pool(name="ps_u", bufs=2, space="PSUM") as ps_u,
            ):
                ngroups = (T + cg - 1) // cg

                def emit_scores(g, qb, et, grp):
                    qsl = slice(QB * qb, QB * (qb + 1))
                    gsz = min(cg, T - cg * grp)
                    # l-major layout: each concurrently-executing row-tiled
                    # matmul of the head pair writes its own PSUM bank
                    # (same-bank concurrent PE writes are a hard fault).
                    ps = ps_s.tile([128, 2, cg, QB], F32, tag="s", name="pss")
                    for tt in range(gsz):
                        t = cg * grp + tt
                        for l in range(2):
                            h = 2 * g + l
                            poff = 32 * (h % 4)
                            nc.tensor.matmul(
                                ps[:, l, tt, :],
                                kt_sb[h // 4][poff : poff + 32, 128 * t : 128 * (t + 1)],
                                qt_sb[h // 4][poff : poff + 32, qsl],
                                start=True,
                                stop=True,
                                tile_position=(poff, 0) if use_tile_position else None,
                            )
                    nc.scalar.activation(
                        out=et[:, cg * grp : cg * grp + gsz, :, :].rearrange(
                            "p t l q -> p l t q"
                        ),
                        in_=ps[:, :, 0:gsz, :],
                        func=mybir.ActivationFunctionType.Exp,
                        scale=SCALE,
                    )

                def emit_pv(item, lo, hi):
                    # slice [lo, hi) of the 2*T-long concatenated (l, t)
                    # accumulation sequence (l=0 fully, then l=1)
                    g, qb, et, uo = item
                    for i in range(lo, hi):
                        l, t = divmod(i, T)
                        nc.tensor.matmul(
                            uo[64 * l : 64 * l + D + 1, :],
                            v_sb[:, t, 2 * g + l, :],
                            et[:, t, l, :],
                            start=(t == 0),
                            stop=(t == T - 1),
                        )

                def emit_normalize(item):
                    g, qb, et, uo = item
                    qsl = slice(QB * qb, QB * (qb + 1))
                    for l in range(2):
                        h = 2 * g + l
                        r = r_pool.tile([1, QB], F32, tag="r", name="r")
                        nc.vector.reciprocal(
                            out=r, in_=uo[64 * l + D : 64 * l + D + 1, :]
                        )
                        R = R_pool.tile([D, QB], F32, tag="R", name="R")
                        if bcast == "gpsimd":
                            nc.gpsimd.partition_broadcast(R, r)
                        else:
                            r_ap = r[0:1, :]
                            rb = bass.AP(
                                tensor=r_ap.tensor,
                                offset=r_ap.offset,
                                ap=[[0, D]] + list(r_ap.ap[1:]),
                            )
                            nc.sync.dma_start(out=R, in_=rb)
                        osl = ot_sb[h // 4][32 * (h % 4) : 32 * (h % 4 + 1), qsl]
                        nc.vector.tensor_mul(
                            out=osl, in0=uo[64 * l : 64 * l + D, :], in1=R
                        )
                        nc.vector.tensor_scalar_add(
                            out=osl,
                            in0=osl,
                            scalar1=b_sb[
                                32 * (h % 4) : 32 * (h % 4 + 1), 4 + h // 4 : 5 + h // 4
                            ],
                        )

                def emit_proj(qb):
                    qsl = slice(QB * qb, QB * (qb + 1))
                    for m in range(2):
                        ps = ps_u.tile([128, QB], F32, tag="u", name="psp")
                        for k in range(2):
                            nc.tensor.matmul(
                                ps,
                                wp_sb[k][:, 128 * m : 128 * (m + 1)],
                                ot_sb[k][:, qsl],
                                start=(k == 0),
                                stop=(k == 1),
                            )
                        po = po_pool.tile([128, QB], F32, tag="po", name="po")
                        nc.vector.tensor_scalar_add(
                            out=po, in0=ps, scalar1=bp_sb[:, m : m + 1]
                        )
                        nc.sync.dma_start(
                            out=out_d[128 * m : 128 * (m + 1), qsl], in_=po
                        )

                def retire(item):
                    emit_normalize(item)
                    if item[0] == H // 2 - 1:  # last pair of this query block
                        emit_proj(item[1])

                pending = None
                pv_per_grp = -(-2 * T // ngroups)
                for qb in range(NQB):
                    for g in range(H // 2):
                        et = big.tile([128, T, 2, QB], F16, tag="big", name="et")
                        for grp in range(ngroups):
                            emit_scores(g, qb, et, grp)
                            if pending is not None:
                                emit_pv(
                                    pending,
                                    pv_per_grp * grp,
                                    min(pv_per_grp * (grp + 1), 2 * T),
                                )
                        if pending is not None:
                            retire(pending)
                        uo = ps_u.tile([128, QB], F32, tag="u", name="psu")
                        pending = (g, qb, et, uo)
                # drain the last item
                emit_pv(pending, 0, 2 * T)
                retire(pending)

    nc.compile()
    return nc


def _make_in_maps(x, w_qkv, b_qkv, w_proj, b_proj, n_cores=8):
    B, Cx, Hi, Wi = x.shape
    S = Hi * Wi
    NQ = S * B // n_cores
    xr = np.ascontiguousarray(x.reshape(B, Cx, S).astype(np.float32))
    wqkvT = np.ascontiguousarray(w_qkv.astype(np.float32).T)
    bqkv = np.ascontiguousarray(b_qkv.astype(np.float32).reshape(3 * Cx, 1))
    wpT = np.ascontiguousarray(w_proj.astype(np.float32).T)
    bp = np.ascontiguousarray(b_proj.astype(np.float32).reshape(Cx, 1))
    halves = n_cores // B
    in_maps = []
    for core in range(n_cores):
        b, half = divmod(core, halves)
        xc = xr[b]
        xq = np.ascontiguousarray(xc[:, half * NQ : (half + 1) * NQ])
        in_maps.append(
            {"xc": xc, "xq": xq, "wqkvT": wqkvT, "bqkv": bqkv, "wpT": wpT, "bp": bp}
        )
    return in_maps, (B, Cx, Hi, Wi, S, NQ)


_NC_CACHE = {}


def run(x, w_qkv, b_qkv, w_proj, b_proj, trace=False, **spmd_kwargs):
    from concourse.bass_utils import run_bass_kernel_spmd

    in_maps, (B, Cx, Hi, Wi, S, NQ) = _make_in_maps(x, w_qkv, b_qkv, w_proj, b_proj)
    key = (S, NQ)
    if key not in _NC_CACHE:
        _NC_CACHE[key] = build_nc(S, NQ)
    nc = _NC_CACHE[key]
    res = run_bass_kernel_spmd(
        nc, in_maps, core_ids=list(range(8)), trace=trace, **spmd_kwargs
    )
    outs = [r["out"] for r in res.results]
    halves = 8 // B
    full = np.empty((B, Cx, S), np.float32)
    for b in range(B):
        for half in range(halves):
            full[b, :, half * NQ : (half + 1) * NQ] = outs[halves * b + half]
    return full.reshape(B, Cx, Hi, Wi), res


def kernel(x, w_qkv, b_qkv, w_proj, b_proj):
    out, _ = run(x, w_qkv, b_qkv, w_proj, b_proj)
    return out



# revision 8
# speedup vs baseline: 1.3037x; 1.3037x over previous
"""Multi-head self-attention (1x1-conv QKV + attention + 1x1-conv proj) for
Trainium2, distributed over 8 NeuronCores.

Problem (hardcoded shapes):
  x: (4, 256, 64, 64) f32;  DIM=256, heads=8, head_dim=32, S=64*64=4096.

Sharding: 8 cores = 4 batches x 2 query-halves. Every core holds the full
key/value sequence for its batch and computes attention + projection for its
half of the queries (NQ=2048). The host concatenates per-core outputs.

Per-core design (v2 — dual-engine softmax):
  - exp() is the throughput wall (S*NQ*H = 67M exps/core, scalar engine only).
    This kernel splits the exp work between the scalar engine (true exp LUT)
    and the vector engine, which computes a Schraudolph-style approximate
    exp2 via one fused tensor_scalar: i16 = round(1024*(s*scale*log2e +
    (15-c))) bit-viewed as fp16. Max rel err ~3% on the approximated tiles;
    softmax normalization cancels most of it (measured end-to-end ~6e-3,
    gate is 2e-2).
  - Scores are computed keys-on-partitions (S^T = K^T q) so E^T feeds the
    P@V matmul directly; the softmax denominator rides as a ones-column in V.
  - Loop order (qp, head-pair, t): the kt stationary is reused across two
    512-query chunks per LDW; score matmuls run 2-way row-tiled (quadrant
    pairs), P@V runs 2-way column-tiled (head l=0 at PSUM rows 0:33, l=1 at
    64:97).
  - P@V accumulation interleaves l0/l1 chains into one PSUM bank; l1 leads
    by one chunk so its start=True bank-bit-clear never erases l0 state.
  - v-bias is folded into V at setup (denominator multiplies it back out);
    q/k biases ride the QKV bias-add; reciprocal uses the fast approx DVE op.
"""

import numpy as np

import concourse.bacc as bacc
import concourse.bass as bass
import concourse.tile as tile
from concourse import mybir

C = 256  # channels
HN = 8  # heads
D = 32  # head dim
F32 = mybir.dt.float32
F16 = mybir.dt.float16
I16 = mybir.dt.int16

SCALE = float(D) ** -0.5
SCH_C = 0.045  # Schraudolph minimax constant
A16 = float(1024.0 * SCALE * np.log2(np.e))
B16 = float(1024.0 * (15.0 - SCH_C))


def build_nc(S, NQ, act_frac=0.563, pv_lag=2):
    """Single-core SPMD program. S: keys, NQ: queries this core handles."""
    assert S % 128 == 0 and NQ % 1024 == 0
    T = S // 128  # key chunks
    QP = NQ // 1024  # query groups (1024 queries each)

    nc = bacc.Bacc("TRN2", target_bir_lowering=False)

    xc_d = nc.dram_tensor("xc", [C, S], F16, kind="ExternalInput")
    xq_d = nc.dram_tensor("xq", [C, NQ], F16, kind="ExternalInput")
    wqkvT_d = nc.dram_tensor("wqkvT", [C, 3 * C], F16, kind="ExternalInput")
    bqkv_d = nc.dram_tensor("bqkv", [3 * C, 1], F32, kind="ExternalInput")
    wpT_d = nc.dram_tensor("wpT", [C, C], F16, kind="ExternalInput")
    bp_d = nc.dram_tensor("bp", [C, 1], F32, kind="ExternalInput")
    out_d = nc.dram_tensor("out", [C, NQ], F32, kind="ExternalOutput")

    with tile.TileContext(nc) as tc:
        with (
            tc.tile_pool(name="per", bufs=1) as per,
            tc.tile_pool(name="et", bufs=6) as et_pool,
            tc.tile_pool(name="ps", bufs=3, space="PSUM") as ps_pool,
            tc.tile_pool(name="uo", bufs=2, space="PSUM") as uo_pool,
            tc.tile_pool(name="r", bufs=4) as r_pool,
            tc.tile_pool(name="R", bufs=4) as R_pool,
            tc.tile_pool(name="po", bufs=4) as po_pool,
        ):
            # ---- persistent SBUF tiles ----
            scratch = per.tile([1, 8], F32, tag="scr", name="scratch")
            xc_sb = [per.tile([128, S], F16, tag=f"xc{k}", name=f"xc{k}") for k in range(2)]
            xq_sb = [per.tile([128, NQ], F16, tag=f"xq{k}", name=f"xq{k}") for k in range(2)]
            w_sb = [per.tile([128, 3 * C], F16, tag=f"w{k}", name=f"w{k}") for k in range(2)]
            wp_sb = [per.tile([128, C], F16, tag=f"wp{k}", name=f"wp{k}") for k in range(2)]
            b_sb = per.tile([128, 4], F32, tag="b", name="b")  # bq0 bq1 bk0 bk1
            bp_sb = per.tile([128, 2], F32, tag="bp", name="bp")
            bv_row = per.tile([1, C], F32, tag="bvr", name="bvr")
            bv_sb = per.tile([128, C], F32, tag="bv", name="bv")
            qt_sb = [per.tile([128, NQ], F16, tag=f"qt{k}", name=f"qt{k}") for k in range(2)]
            kt_sb = [per.tile([128, S], F16, tag=f"kt{k}", name=f"kt{k}") for k in range(2)]
            v_sb = per.tile([128, T, HN, D + 1], F16, tag="v", name="v")
            ot_sb = [per.tile([128, NQ], F16, tag=f"ot{k}", name=f"ot{k}") for k in range(2)]

            # preload the exp table before any other ACT op
            nc.vector.memset(scratch, 0.0)
            nc.scalar.activation(
                out=scratch[:, 4:8], in_=scratch[:, 0:4],
                func=mybir.ActivationFunctionType.Exp,
            )

            # ---- input DMA ----
            for k in range(2):
                nc.sync.dma_start(out=w_sb[k], in_=wqkvT_d[128 * k : 128 * (k + 1), :])
                nc.sync.dma_start(out=wp_sb[k], in_=wpT_d[128 * k : 128 * (k + 1), :])
                nc.sync.dma_start(out=xq_sb[k], in_=xq_d[128 * k : 128 * (k + 1), :])
                nc.sync.dma_start(
                    out=bp_sb[:, k : k + 1], in_=bp_d[128 * k : 128 * (k + 1), :]
                )
            for j in range(4):
                nc.sync.dma_start(
                    out=b_sb[:, j : j + 1], in_=bqkv_d[128 * j : 128 * (j + 1), :]
                )
            nc.sync.dma_start(
                out=bv_row, in_=bqkv_d[2 * C : 3 * C, 0:1].rearrange("a b -> b a")
            )
            nc.gpsimd.partition_broadcast(bv_sb, bv_row)
            for k in range(2):
                nc.sync.dma_start(out=xc_sb[k], in_=xc_d[128 * k : 128 * (k + 1), :])

            # ones column for the softmax-denominator trick
            nc.vector.memset(v_sb[:, :, :, D : D + 1], 1.0)

            # ---- QKV projections ----
            # Q^T / K^T: (c' on partitions, seq on free), bias-add on ACT.
            def qk_setup(which, src_sb, dst_sb, ncol, bcol):
                for kk in range(2):
                    for npair in range(ncol // 1024):
                        ps = ps_pool.tile([128, 2, 512], F32, tag="ps", name="psqk")
                        for j in range(2):
                            n = 2 * npair + j
                            for k in range(2):
                                nc.tensor.matmul(
                                    ps[:, j, :],
                                    w_sb[k][:, 256 * which + 128 * kk : 256 * which + 128 * (kk + 1)],
                                    src_sb[k][:, 512 * n : 512 * (n + 1)],
                                    start=(k == 0),
                                    stop=(k == 1),
                                )
                        nc.scalar.add(
                            out=dst_sb[kk][:, 1024 * npair : 1024 * (npair + 1)].rearrange(
                                "p (j n) -> p j n", j=2
                            ),
                            in_=ps,
                            add=b_sb[:, bcol + kk : bcol + kk + 1],
                        )

            qk_setup(0, xq_sb, qt_sb, NQ, 0)  # Q^T
            # K^T in two halves so V production can interleave; kk=0 first
            # (first head-pairs only need kt_sb[0]).
            def kt_half(kk):
                for npair in range(S // 1024):
                    ps = ps_pool.tile([128, 2, 512], F32, tag="ps", name="pskt")
                    for j in range(2):
                        n = 2 * npair + j
                        for k in range(2):
                            nc.tensor.matmul(
                                ps[:, j, :],
                                w_sb[k][:, 256 + 128 * kk : 256 + 128 * (kk + 1)],
                                xc_sb[k][:, 512 * n : 512 * (n + 1)],
                                start=(k == 0),
                                stop=(k == 1),
                            )
                    nc.scalar.add(
                        out=kt_sb[kk][:, 1024 * npair : 1024 * (npair + 1)].rearrange(
                            "p (j n) -> p j n", j=2
                        ),
                        in_=ps,
                        add=b_sb[:, 2 + kk : 3 + kk],
                    )

            kt_half(0)
            # V in natural layout (keys on partitions): (128, t, h, d) f16,
            # with the v-bias folded in (denominator divides it back out).
            for tq in range(T // 4):
                ps4 = ps_pool.tile([128, 4, 256], F32, tag="ps", name="psv")
                for j in range(4):
                    t = 4 * tq + j
                    for k in range(2):
                        nc.tensor.matmul(
                            ps4[:, j, :],
                            xc_sb[k][:, 128 * t : 128 * (t + 1)],
                            w_sb[k][:, 512:768],
                            start=(k == 0),
                            stop=(k == 1),
                        )
                nc.vector.tensor_add(
                    out=v_sb[:, 4 * tq : 4 * (tq + 1), :, 0:D],
                    in0=ps4.rearrange("p j (h d) -> p j h d", h=HN),
                    in1=bv_sb.rearrange("p (h d) -> p h d", h=HN)
                    .unsqueeze(1)
                    .broadcast_to([128, 4, HN, D]),
                )
            kt_half(1)

            # ---- attention ----
            owner_acc = [0.0]

            def next_owner():
                owner_acc[0] += act_frac
                if owner_acc[0] >= 1.0:
                    owner_acc[0] -= 1.0
                    return "act"
                return "dve"

            for qp in range(QP):
                qoff = 1024 * qp
                for hp in range(HN // 2):
                    kk = hp // 2
                    h0 = 2 * hp
                    poffs = (32 * (h0 % 4), 32 * ((h0 + 1) % 4))
                    uo = [
                        uo_pool.tile([128, 512], F32, tag="u", name=f"uo{qc}")
                        for qc in range(2)
                    ]
                    et_hist = {}

                    def emit_pv_wave(w):
                        # PSUM has_written bits are region-scoped: each l
                        # chain clears its own partition rows with start=True.
                        for qc in range(2):
                            for l in range(2):
                                nc.tensor.matmul(
                                    uo[qc][64 * l : 64 * l + D + 1, :],
                                    v_sb[:, w, h0 + l, :],
                                    et_hist[w][:, l, 512 * qc : 512 * (qc + 1)],
                                    start=(w == 0),
                                    stop=(w == T - 1),
                                    tile_position=(0, 64 * l),
                                )
                        del et_hist[w]

                    for t in range(T):
                        et = et_pool.tile([128, 2, 1024], F16, tag="et", name="et")
                        et_hist[t] = et
                        pss = [
                            ps_pool.tile([128, 2, 512], F32, tag="ps", name="pss")
                            for _ in range(2)
                        ]
                        for qc in range(2):
                            for l in range(2):
                                nc.tensor.matmul(
                                    pss[qc][:, l, :],
                                    kt_sb[kk][
                                        poffs[l] : poffs[l] + D, 128 * t : 128 * (t + 1)
                                    ],
                                    qt_sb[kk][
                                        poffs[l] : poffs[l] + D,
                                        qoff + 512 * qc : qoff + 512 * (qc + 1),
                                    ],
                                    start=True,
                                    stop=True,
                                    tile_position=(poffs[l], 0),
                                )
                        for qc in range(2):
                            dst = et[:, :, 512 * qc : 512 * (qc + 1)]
                            if next_owner() == "act":
                                nc.scalar.activation(
                                    out=dst,
                                    in_=pss[qc],
                                    func=mybir.ActivationFunctionType.Exp,
                                    scale=SCALE,
                                )
                            else:
                                nc.vector.tensor_scalar(
                                    out=dst.bitcast(I16),
                                    in0=pss[qc],
                                    scalar1=A16,
                                    scalar2=B16,
                                    op0=mybir.AluOpType.mult,
                                    op1=mybir.AluOpType.add,
                                )
                        if t >= pv_lag:
                            emit_pv_wave(t - pv_lag)
                    for w in range(T - pv_lag, T):
                        emit_pv_wave(w)

                    # ---- normalize: ot = uo * (1/den), den rows 32 & 96 ----
                    # (R tiles live at base partition 0: partition_broadcast
                    # cannot write at a partition offset; DVE pairs operand
                    # lanes by index, so a base-0 in1 works for l=1 too.)
                    for qc in range(2):
                        Rs = []
                        for l in range(2):
                            # den row sits at partition 32/96; engines can
                            # only write partition-0-based [1,*] tiles and
                            # reciprocal_approx_fast ignores the input base
                            # partition, so copy the row down first.
                            dn = r_pool.tile([1, 512], F32, tag="dn", name="dn")
                            nc.vector.tensor_copy(
                                out=dn, in_=uo[qc][64 * l + D : 64 * l + D + 1, :]
                            )
                            r = r_pool.tile([1, 512], F32, tag="r", name="r")
                            nc.vector.reciprocal_approx_fast(out=r, in_=dn)
                            R = R_pool.tile([D, 512], F32, tag="R", name="R")
                            nc.gpsimd.partition_broadcast(R, r)
                            Rs.append(R)
                        for l in range(2):
                            h = h0 + l
                            nc.vector.tensor_mul(
                                out=ot_sb[kk][
                                    32 * (h % 4) : 32 * (h % 4) + D,
                                    qoff + 512 * qc : qoff + 512 * (qc + 1),
                                ],
                                in0=uo[qc][64 * l : 64 * l + D, :],
                                in1=Rs[l],
                            )

                # ---- projection for this query group ----
                for m in range(2):
                    for qc in range(2):
                        pp = uo_pool.tile([128, 512], F32, tag="u", name="pp")
                        for k in range(2):
                            nc.tensor.matmul(
                                pp,
                                wp_sb[k][:, 128 * m : 128 * (m + 1)],
                                ot_sb[k][:, qoff + 512 * qc : qoff + 512 * (qc + 1)],
                                start=(k == 0),
                                stop=(k == 1),
                            )
                        po = po_pool.tile([128, 512], F32, tag="po", name="po")
                        nc.scalar.add(out=po, in_=pp, add=bp_sb[:, m : m + 1])
                        nc.sync.dma_start(
                            out=out_d[
                                128 * m : 128 * (m + 1),
                                qoff + 512 * qc : qoff + 512 * (qc + 1),
                            ],
                            in_=po,
                        )

    nc.compile()
    return nc


def _make_in_maps(x, w_qkv, b_qkv, w_proj, b_proj, n_cores=8):
    B, Cx, Hi, Wi = x.shape
    S = Hi * Wi
    NQ = S * B // n_cores
    xr = np.ascontiguousarray(x.reshape(B, Cx, S).astype(np.float16))
    wqkvT = np.ascontiguousarray(w_qkv.astype(np.float16).T)
    bqkv = np.ascontiguousarray(b_qkv.astype(np.float32).reshape(3 * Cx, 1))
    wpT = np.ascontiguousarray(w_proj.astype(np.float16).T)
    bp = np.ascontiguousarray(b_proj.astype(np.float32).reshape(Cx, 1))
    halves = n_cores // B
    in_maps = []
    for core in range(n_cores):
        b, half = divmod(core, halves)
        xc = xr[b]
        xq = np.ascontiguousarray(xc[:, half * NQ : (half + 1) * NQ])
        in_maps.append(
            {"xc": xc, "xq": xq, "wqkvT": wqkvT, "bqkv": bqkv, "wpT": wpT, "bp": bp}
        )
    return in_maps, (B, Cx, Hi, Wi, S, NQ)


_NC_CACHE = {}


def run(x, w_qkv, b_qkv, w_proj, b_proj, trace=False, build_kwargs=None, **spmd_kwargs):
    from concourse.bass_utils import run_bass_kernel_spmd

    in_maps, (B, Cx, Hi, Wi, S, NQ) = _make_in_maps(x, w_qkv, b_qkv, w_proj, b_proj)
    key = (S, NQ, tuple(sorted((build_kwargs or {}).items())))
    if key not in _NC_CACHE:
        _NC_CACHE[key] = build_nc(S, NQ, **(build_kwargs or {}))
    nc = _NC_CACHE[key]
    res = run_bass_kernel_spmd(
        nc, in_maps, core_ids=list(range(8)), trace=trace, **spmd_kwargs
    )
    outs = [r["out"] for r in res.results]
    halves = 8 // B
    full = np.empty((B, Cx, S), np.float32)
    for b in range(B):
        for half in range(halves):
            full[b, :, half * NQ : (half + 1) * NQ] = outs[halves * b + half]
    return full.reshape(B, Cx, Hi, Wi), res


def kernel(x, w_qkv, b_qkv, w_proj, b_proj):
    out, _ = run(x, w_qkv, b_qkv, w_proj, b_proj)
    return out
